# revision 19
# baseline (speedup 1.0000x reference)
"""Fused pre-LN multi-head attention block for Trainium2, sharded over 8 NeuronCores.

Sharding: batch x head-group tensor parallel. Core c handles batch b=c//4 and
head group g=c%4 (4 heads of 64 dims = 256 columns of Wq/Wk/Wv, 256 rows of Wo).
Each core computes LayerNorm(x_b) (gamma/beta folded into weights host-side),
QKV for its heads, attention, and a partial output projection. The host sums
the 4 partials per batch and adds bias + residual (the attention branch is tiny
next to the residual, so bf16 matmul inputs cost ~1e-5 scale-relative error).

Device pipeline (per core):
  pass A: load x tiles (kept resident), row stats, batched rstd=exp(-.5*ln(var+eps))
  pass B, per 512-col chunk of S: normalize to bf16, DMA-xbar transpose into
    zTc [H part, 512], then QKV matmuls (q/k transposed layout [256 part, S],
    v natural [S part, 256]).
  attention, per (S_q half, head pair): per S_k tile j, row-tiled pair matmul
    -> scoresT [128, half] PSUM -> exp on ACT (mask/scale folded in) -> probsT
    bf16 SBUF; col-tiled PV accumulates ctxT in PSUM; ones-matmul row sums;
    softmax normalization folded into ctx eviction via partition_broadcast.
  output projection: ctxT^T @ woT -> partial out [S, H] fp32.
"""

import os
import sys

sys.path.insert(0, "/opt/trn_rl_repo")

import numpy as np
import ml_dtypes

import concourse.bacc as bacc
import concourse.bass as bass
import concourse.mybir as mybir
from concourse import tile

F32 = mybir.dt.float32
BF16 = mybir.dt.bfloat16
AF = mybir.ActivationFunctionType
ALU = mybir.AluOpType

H = 1024
NHEADS = 16
HD = 64
DG = 256  # head dims per core (4 heads x 64)
NCORES = 8
EPS = 1e-12


def build_program(S=2048, debug_outs=False):
    nc = bacc.Bacc(
        "TRN2", target_bir_lowering=False, debug=False, num_devices=NCORES
    )
    x_d = nc.dram_tensor("x", [S, H], F32, kind="ExternalInput").ap()
    wqT_d = nc.dram_tensor("wqT", [H, DG], BF16, kind="ExternalInput").ap()
    wkT_d = nc.dram_tensor("wkT", [H, DG], BF16, kind="ExternalInput").ap()
    wvT_d = nc.dram_tensor("wvT", [H, DG], BF16, kind="ExternalInput").ap()
    woT_d = nc.dram_tensor("woT", [DG, H], BF16, kind="ExternalInput").ap()
    bq_d = nc.dram_tensor("bq", [128, 2], F32, kind="ExternalInput").ap()
    bk_d = nc.dram_tensor("bk", [128, 2], F32, kind="ExternalInput").ap()
    bv_d = nc.dram_tensor("bv", [128, DG], F32, kind="ExternalInput").ap()
    mask_d = nc.dram_tensor("mask", [128, S // 128], F32, kind="ExternalInput").ap()
    out_d = nc.dram_tensor("out", [S, H], F32, kind="ExternalOutput").ap()
    if debug_outs:
        dbg_zT = nc.dram_tensor("dbg_zT", [128, H // 128, S], BF16, kind="ExternalOutput").ap()
        dbg_qT = nc.dram_tensor("dbg_qT", [128, 2, S], BF16, kind="ExternalOutput").ap()
        dbg_kT = nc.dram_tensor("dbg_kT", [128, 2, S], BF16, kind="ExternalOutput").ap()
        dbg_vN = nc.dram_tensor("dbg_vN", [128, S // 128, DG], BF16, kind="ExternalOutput").ap()
        dbg_cT = nc.dram_tensor("dbg_cT", [128, 2, S], BF16, kind="ExternalOutput").ap()
        dbg_pr = nc.dram_tensor("dbg_pr", [128, S // 2], BF16, kind="ExternalOutput").ap()
        dbg_sums = nc.dram_tensor("dbg_sums", [128, 512], F32, kind="ExternalOutput").ap()
        dbg_recip = nc.dram_tensor("dbg_recip", [128, 512], F32, kind="ExternalOutput").ap()
        dbg_rb = nc.dram_tensor("dbg_rb", [128, S // 2], F32, kind="ExternalOutput").ap()
        dbg_ctxps = nc.dram_tensor("dbg_ctxps", [128, S // 2], F32, kind="ExternalOutput").ap()

    NT = S // 128  # S tiles
    KT = H // 128  # 8
    HALF = S // 2  # S_q half width
    CH = min(512, HALF)  # S_q chunk (one PSUM bank)
    NCH = HALF // CH  # chunks per half
    CW = min(512, S)  # column chunk for pass B
    NC2 = S // CW
    CWT = CW // 128  # S tiles per chunk

    with tile.TileContext(nc) as tc:
        with (
            tc.tile_pool(name="const", bufs=1) as constp,
            tc.tile_pool(name="big", bufs=1) as bigp,
            tc.tile_pool(name="xin", bufs=1) as xinp,
            tc.tile_pool(name="work", bufs=3) as workp,
            tc.tile_pool(name="probs", bufs=3) as probsp,
            tc.tile_pool(name="psA", bufs=2, space="PSUM") as psA,
            tc.tile_pool(name="psB", bufs=1, space="PSUM") as psB,
        ):
            ones_f = constp.tile([128, 64], F32)
            nc.gpsimd.memset(ones_f, 1.0)
            ones = constp.tile([128, 64], BF16)
            nc.vector.tensor_copy(ones, ones_f)
            eps_b = constp.tile([128, 1], F32)
            nc.gpsimd.memset(eps_b, EPS)
            mask_sb = constp.tile([128, NT], F32)
            nc.sync.dma_start(mask_sb, mask_d)
            bq_sb = constp.tile([128, 2], F32)
            nc.sync.dma_start(bq_sb, bq_d)
            bk_sb = constp.tile([128, 2], F32)
            nc.sync.dma_start(bk_sb, bk_d)
            bv_sb = constp.tile([128, DG], F32)
            nc.sync.dma_start(bv_sb, bv_d)

            wq_sb = bigp.tile([128, KT, DG], BF16)
            nc.sync.dma_start(wq_sb, wqT_d.rearrange("(k p) d -> p k d", p=128))
            wk_sb = bigp.tile([128, KT, DG], BF16)
            nc.sync.dma_start(wk_sb, wkT_d.rearrange("(k p) d -> p k d", p=128))
            wv_sb = bigp.tile([128, KT, DG], BF16)
            nc.sync.dma_start(wv_sb, wvT_d.rearrange("(k p) d -> p k d", p=128))
            wo_sb = bigp.tile([128, DG // 128, H], BF16)
            nc.sync.dma_start(wo_sb, woT_d.rearrange("(k p) d -> p k d", p=128))

            qT = bigp.tile([128, 2, S], BF16)
            kTt = bigp.tile([128, 2, S], BF16)
            vN = bigp.tile([128, NT, DG], BF16)
            cT = bigp.tile([128, 2, S], BF16)
            mv_all = bigp.tile([128, NT, 2], F32)
            rstd_all = bigp.tile([128, NT], F32)

            # ---- pass A: load x (resident), row stats (mean/var over H) ----
            xts = []
            for i in range(NT):
                xt = xinp.tile([128, H], F32, tag=f"xt{i}", bufs=1)
                nc.sync.dma_start(xt, x_d[i * 128 : (i + 1) * 128, :])
                st = workp.tile([128, 2, 6], F32, tag="st")
                for a in range(2):
                    nc.vector.bn_stats(st[:, a, :], xt[:, a * 512 : (a + 1) * 512])
                nc.vector.bn_aggr(mv_all[:, i, :], st)
                xts.append(xt)
            # rstd = exp(-0.5 * ln(var + eps)); Ln/Exp batched -> few table loads
            lnv = workp.tile([128, NT], F32, tag="lnv")
            nc.scalar.activation(lnv, mv_all[:, :, 1], AF.Ln, bias=eps_b)
            nc.scalar.activation(rstd_all, lnv, AF.Exp, scale=-0.5)

            # ---- pass B: per column-chunk: normalize, transpose, QKV ----
            with tc.tile_pool(name="ph12", bufs=2) as zpool:
                for n in range(NC2):
                    zTc = zpool.tile([128, KT, CW], BF16, tag="zTc")
                    for i4 in range(CWT):
                        i = n * CWT + i4
                        zt = workp.tile([128, H], BF16, tag="zt", bufs=2)
                        nc.vector.tensor_scalar(
                            zt, xts[i], mv_all[:, i, 0:1], rstd_all[:, i : i + 1],
                            ALU.subtract, ALU.mult,
                        )
                        nc.sync.dma_start_transpose(
                            zTc[:, :, i4 * 128 : (i4 + 1) * 128], zt
                        )
                    if debug_outs:
                        nc.sync.dma_start(dbg_zT[:, :, n * CW : (n + 1) * CW], zTc)
                    # q/k for this chunk (transposed layout)
                    for tout, wsb, bsb in ((qT, wq_sb, bq_sb), (kTt, wk_sb, bk_sb)):
                        for m in range(2):
                            ps = psA.tile([128, CW], F32, tag="stage", bufs=1)
                            for kk in range(KT):
                                nc.tensor.matmul(
                                    ps,
                                    wsb[:, kk, m * 128 : (m + 1) * 128],
                                    zTc[:, kk, :],
                                    start=(kk == 0),
                                    stop=(kk == KT - 1),
                                )
                            nc.vector.tensor_scalar_add(
                                tout[:, m, n * CW : (n + 1) * CW], ps,
                                bsb[:, m : m + 1],
                            )
                    # v for this chunk (natural layout)
                    for i4 in range(CWT):
                        i = n * CWT + i4
                        ps = psA.tile([128, DG], F32, tag="stage", bufs=1)
                        for kk in range(KT):
                            nc.tensor.matmul(
                                ps,
                                zTc[:, kk, i4 * 128 : (i4 + 1) * 128],
                                wv_sb[:, kk, :],
                                start=(kk == 0),
                                stop=(kk == KT - 1),
                            )
                        nc.vector.tensor_tensor(vN[:, i, :], ps, bv_sb, ALU.add)

            if debug_outs:
                nc.sync.dma_start(dbg_qT, qT)
                nc.sync.dma_start(dbg_kT, kTt)
                nc.sync.dma_start(dbg_vN, vN)

            # ---- attention ----
            for sH in range(2):
                sq0 = sH * HALF
                for p in range(2):  # head pair = M-tile of qT/kT
                    ctx_ps = psB.tile([128, HALF], F32, tag="ctx")
                    sums_ps = psB.tile([128, 512], F32, tag="sums")
                    for j in range(NT):
                        prs = []
                        for h in range(2):
                            sc = psA.tile([128, HALF], F32, tag="sc", bufs=2)
                            for c in range(NCH):
                                nc.tensor.matmul(
                                    sc[:, c * CH : (c + 1) * CH],
                                    kTt[64 * h : 64 * h + 64, p,
                                        j * 128 : (j + 1) * 128],
                                    qT[64 * h : 64 * h + 64, p,
                                       sq0 + c * CH : sq0 + (c + 1) * CH],
                                    tile_position=(64 * h, 0),
                                    start=True,
                                    stop=True,
                                )
                            pr = probsp.tile([128, HALF], BF16, tag=f"pr{h}")
                            nc.scalar.activation(
                                pr, sc, AF.Exp, bias=mask_sb[:, j : j + 1],
                                scale=0.125,
                            )
                            prs.append(pr)
                            if debug_outs and sH == 0 and p == 0 and j == 0 and h == 0:
                                nc.sync.dma_start(dbg_pr, pr)
                        for c in range(NCH):
                            for h in range(2):
                                nc.tensor.matmul(
                                    ctx_ps[64 * h : 64 * h + 64,
                                           c * CH : (c + 1) * CH],
                                    vN[:, j, 64 * (2 * p + h) :
                                       64 * (2 * p + h) + 64],
                                    prs[h][:, c * CH : (c + 1) * CH],
                                    tile_position=(0, 64 * h),
                                    start=(j == 0),
                                    stop=(j == NT - 1),
                                    skip_group_check=True,
                                )
                        for c in range(NCH):
                            for h in range(2):
                                pos = 64 * c + 32 * h
                                nc.tensor.matmul(
                                    sums_ps[pos : pos + 1, 0:CH],
                                    ones[:, 0:1],
                                    prs[h][:, c * CH : (c + 1) * CH],
                                    tile_position=(0, pos),
                                    start=(j == 0),
                                    stop=(j == NT - 1),
                                    skip_group_check=True,
                                )
                    if debug_outs and sH == 0 and p == 0:
                        sums_sb = workp.tile([128, 512], F32, tag="dbgs", bufs=1)
                        ctx_sb = workp.tile([128, HALF], F32, tag="dbgc", bufs=1)
                        for c in range(NCH):
                            for h in range(2):
                                pos = 64 * c + 32 * h
                                nc.vector.tensor_copy(
                                    sums_sb[pos : pos + 1, 0:CH],
                                    sums_ps[pos : pos + 1, 0:CH],
                                )
                        nc.vector.tensor_copy(ctx_sb, ctx_ps)
                        nc.sync.dma_start(dbg_sums, sums_sb)
                        nc.sync.dma_start(dbg_ctxps, ctx_sb)
                    sums_sb = workp.tile([128, 512], F32, tag="sums_sb", bufs=2)
                    recip_f = workp.tile([128, 512], F32, tag="recip_f", bufs=2)
                    recip = workp.tile([128, 512], BF16, tag="recip", bufs=2)
                    with nc.allow_low_precision("softmax recip in bf16"):
                        for c in range(NCH):
                            for h in range(2):
                                pos = 64 * c + 32 * h
                                nc.vector.tensor_copy(
                                    sums_sb[pos : pos + 1, 0:CH],
                                    sums_ps[pos : pos + 1, 0:CH],
                                )
                                nc.vector.reciprocal(
                                    recip_f[pos : pos + 1, 0:CH],
                                    sums_sb[pos : pos + 1, 0:CH],
                                )
                                nc.vector.tensor_copy(
                                    recip[pos : pos + 1, 0:CH],
                                    recip_f[pos : pos + 1, 0:CH],
                                )
                    rb_ps = psA.tile([128, HALF], F32, tag="sc", bufs=2)
                    for c in range(NCH):
                        for h in range(2):
                            pos = 64 * c + 32 * h
                            nc.tensor.matmul(
                                rb_ps[64 * h : 64 * h + 64, c * CH : (c + 1) * CH],
                                ones[pos : pos + 1, :],
                                recip[pos : pos + 1, 0:CH],
                                tile_position=(pos, 64 * h),
                                start=True,
                                stop=True,
                                skip_group_check=True,
                            )
                    rb = workp.tile([128, HALF], F32, tag="rb", bufs=2)
                    nc.vector.tensor_copy(rb, rb_ps)
                    if debug_outs and sH == 0 and p == 0:
                        rcf = workp.tile([128, 512], F32, tag="dbgr", bufs=1)
                        for c in range(NCH):
                            for h in range(2):
                                pos = 64 * c + 32 * h
                                nc.vector.tensor_copy(
                                    rcf[pos : pos + 1, 0:CH],
                                    recip_f[pos : pos + 1, 0:CH],
                                )
                        nc.sync.dma_start(dbg_recip, rcf)
                        nc.sync.dma_start(dbg_rb, rb)
                    nc.vector.tensor_tensor(
                        cT[:, p, sq0 : sq0 + HALF], ctx_ps, rb, ALU.mult
                    )

            if debug_outs:
                nc.sync.dma_start(dbg_cT, cT)

            # ---- output projection (partial) ----
            for i in range(NT):
                ot = workp.tile([128, H], F32, tag="ot", bufs=2)
                for n in range(H // 512):
                    ps = psA.tile([128, 512], F32, tag="stage", bufs=1)
                    for kk in range(DG // 128):
                        nc.tensor.matmul(
                            ps,
                            cT[:, kk, i * 128 : (i + 1) * 128],
                            wo_sb[:, kk, n * 512 : (n + 1) * 512],
                            start=(kk == 0),
                            stop=(kk == DG // 128 - 1),
                        )
                    nc.vector.tensor_copy(ot[:, n * 512 : (n + 1) * 512], ps)
                nc.sync.dma_start(out_d[i * 128 : (i + 1) * 128, :], ot)

    nc.compile()
    return nc


def make_in_maps(hidden_states, attention_mask, wq, bq, wk, bk, wv, bv, wo, bo,
                 ln_gamma, ln_beta, S):
    NT = S // 128
    g32 = np.asarray(ln_gamma).astype(np.float32)
    b32 = np.asarray(ln_beta).astype(np.float32)
    bf = ml_dtypes.bfloat16
    in_maps = []
    for c in range(NCORES):
        b = c // 4
        g = c % 4
        sl = slice(g * DG, (g + 1) * DG)
        wq_sl = np.asarray(wq)[sl, :].astype(np.float32)
        wk_sl = np.asarray(wk)[sl, :].astype(np.float32)
        wv_sl = np.asarray(wv)[sl, :].astype(np.float32)
        m = {
            "x": np.ascontiguousarray(np.asarray(hidden_states)[b], dtype=np.float32),
            "wqT": np.ascontiguousarray((wq_sl * g32[None, :]).T.astype(bf)),
            "wkT": np.ascontiguousarray((wk_sl * g32[None, :]).T.astype(bf)),
            "wvT": np.ascontiguousarray((wv_sl * g32[None, :]).T.astype(bf)),
            "woT": np.ascontiguousarray(
                np.asarray(wo)[:, sl].astype(np.float32).T.astype(bf)
            ),
            "bq": np.ascontiguousarray(
                (wq_sl @ b32 + np.asarray(bq)[sl]).astype(np.float32).reshape(2, 128).T
            ),
            "bk": np.ascontiguousarray(
                (wk_sl @ b32 + np.asarray(bk)[sl]).astype(np.float32).reshape(2, 128).T
            ),
            "bv": np.ascontiguousarray(
                np.broadcast_to(
                    (wv_sl @ b32 + np.asarray(bv)[sl]).astype(np.float32), (128, DG)
                ).copy()
            ),
            "mask": np.ascontiguousarray(
                np.asarray(attention_mask)[b, 0, 0, :]
                .astype(np.float32).reshape(NT, 128).T
            ),
        }
        in_maps.append(m)
    return in_maps


_NC_CACHE = {}


def kernel(hidden_states, attention_mask, wq, bq, wk, bk, wv, bv, wo, bo,
           ln_gamma, ln_beta):
    hidden_states = np.asarray(hidden_states)
    B, S, _ = hidden_states.shape
    if S not in _NC_CACHE:
        _NC_CACHE[S] = build_program(S)
    nc = _NC_CACHE[S]

    in_maps = make_in_maps(
        hidden_states, attention_mask, wq, bq, wk, bk, wv, bv, wo, bo,
        ln_gamma, ln_beta, S,
    )

    from concourse.bass_utils import run_bass_kernel_spmd

    res = run_bass_kernel_spmd(nc, in_maps, list(range(NCORES)))
    parts = [res.results[c]["out"] for c in range(NCORES)]

    out = np.empty((B, S, H), np.float32)
    bo32 = np.asarray(bo).astype(np.float32)
    for b in range(B):
        acc = parts[4 * b].astype(np.float32).copy()
        for g in range(1, 4):
            acc += parts[4 * b + g]
        out[b] = acc + bo32[None, :] + hidden_states[b].astype(np.float32)
    return out


# revision 25
# speedup vs baseline: 12436.0545x; 12436.0545x over previous
"""Fused pre-LN multi-head attention block for Trainium2, sharded over 8 NeuronCores.

Sharding: batch x head-group tensor parallel. Core c handles batch b=c//4 and
head group g=c%4 (4 heads of 64 dims = 256 columns of Wq/Wk/Wv, 256 rows of Wo).
Each core computes LayerNorm(x_b) (gamma/beta folded into weights host-side),
QKV for its heads, attention, and a partial output projection. The host sums
the 4 partials per batch and adds bias + residual (the attention branch is tiny
next to the residual, so bf16 matmul inputs cost ~1e-5 scale-relative error).

Device pipeline (per core):
  pass A: load x tiles (kept resident), row stats, batched rstd=exp(-.5*ln(var+eps))
  pass B, per 512-col chunk of S: normalize to bf16, DMA-xbar transpose into
    zTc [H part, 512], then QKV matmuls (q/k transposed layout [256 part, S],
    v natural [S part, 256]).
  attention, per (S_q half, head pair): per S_k tile j, row-tiled pair matmul
    -> scoresT [128, half] PSUM -> exp on ACT (mask/scale folded in) -> probsT
    bf16 SBUF; col-tiled PV accumulates ctxT in PSUM; ones-matmul row sums;
    softmax normalization folded into ctx eviction via partition_broadcast.
  output projection: ctxT^T @ woT -> partial out [S, H] fp32.
"""

import os
import sys

sys.path.insert(0, "/opt/trn_rl_repo")

import numpy as np
import ml_dtypes

import concourse.bacc as bacc
import concourse.bass as bass
import concourse.mybir as mybir
from concourse import tile

F32 = mybir.dt.float32
BF16 = mybir.dt.bfloat16
AF = mybir.ActivationFunctionType
ALU = mybir.AluOpType

H = 1024
NHEADS = 16
HD = 64
DG = 256  # head dims per core (4 heads x 64)
NCORES = 8
EPS = 1e-12


def build_program(S=2048, debug_outs=False, phases=3):
    nc = bacc.Bacc(
        "TRN2", target_bir_lowering=False, debug=False, num_devices=NCORES
    )
    x_d = nc.dram_tensor("x", [S, H], F32, kind="ExternalInput").ap()
    wqT_d = nc.dram_tensor("wqT", [H, DG], BF16, kind="ExternalInput").ap()
    wkT_d = nc.dram_tensor("wkT", [H, DG], BF16, kind="ExternalInput").ap()
    wvT_d = nc.dram_tensor("wvT", [H, DG], BF16, kind="ExternalInput").ap()
    woT_d = nc.dram_tensor("woT", [DG, H], BF16, kind="ExternalInput").ap()
    bq_d = nc.dram_tensor("bq", [128, 2], F32, kind="ExternalInput").ap()
    bk_d = nc.dram_tensor("bk", [128, 2], F32, kind="ExternalInput").ap()
    bv_d = nc.dram_tensor("bv", [128, DG], F32, kind="ExternalInput").ap()
    mask_d = nc.dram_tensor("mask", [128, S // 128], F32, kind="ExternalInput").ap()
    out_d = nc.dram_tensor("out", [S, H], F32, kind="ExternalOutput").ap()
    if debug_outs:
        dbg_zT = nc.dram_tensor("dbg_zT", [128, H // 128, S], BF16, kind="ExternalOutput").ap()
        dbg_qT = nc.dram_tensor("dbg_qT", [128, 2, S], BF16, kind="ExternalOutput").ap()
        dbg_kT = nc.dram_tensor("dbg_kT", [128, 2, S], BF16, kind="ExternalOutput").ap()
        dbg_vN = nc.dram_tensor("dbg_vN", [128, S // 128, DG], BF16, kind="ExternalOutput").ap()
        dbg_cT = nc.dram_tensor("dbg_cT", [128, 2, S], BF16, kind="ExternalOutput").ap()
        dbg_pr = nc.dram_tensor("dbg_pr", [128, S // 2], BF16, kind="ExternalOutput").ap()
        dbg_sums = nc.dram_tensor("dbg_sums", [128, 512], F32, kind="ExternalOutput").ap()
        dbg_recip = nc.dram_tensor("dbg_recip", [128, 512], F32, kind="ExternalOutput").ap()
        dbg_rb = nc.dram_tensor("dbg_rb", [128, S // 2], F32, kind="ExternalOutput").ap()
        dbg_ctxps = nc.dram_tensor("dbg_ctxps", [128, S // 2], F32, kind="ExternalOutput").ap()

    NT = S // 128  # S tiles
    KT = H // 128  # 8
    HALF = S // 2  # S_q half width
    CH = min(512, HALF)  # S_q chunk (one PSUM bank)
    NCH = HALF // CH  # chunks per half
    CW = min(512, S)  # column chunk for pass B
    NC2 = S // CW
    CWT = CW // 128  # S tiles per chunk

    with tile.TileContext(nc) as tc:
        with (
            tc.tile_pool(name="const", bufs=1) as constp,
            tc.tile_pool(name="big", bufs=1) as bigp,
            tc.tile_pool(name="xin", bufs=1) as xinp,
            tc.tile_pool(name="work", bufs=3) as workp,
            tc.tile_pool(name="probs", bufs=3) as probsp,
            tc.tile_pool(name="psA", bufs=2, space="PSUM") as psA,
            tc.tile_pool(name="psB", bufs=1, space="PSUM") as psB,
        ):
            ones_f = constp.tile([128, 64], F32)
            nc.gpsimd.memset(ones_f, 1.0)
            ones = constp.tile([128, 64], BF16)
            nc.vector.tensor_copy(ones, ones_f)
            eps_b = constp.tile([128, 1], F32)
            nc.gpsimd.memset(eps_b, EPS)
            mask_sb = constp.tile([128, NT], F32)
            nc.sync.dma_start(mask_sb, mask_d)
            bq_sb = constp.tile([128, 2], F32)
            nc.sync.dma_start(bq_sb, bq_d)
            bk_sb = constp.tile([128, 2], F32)
            nc.sync.dma_start(bk_sb, bk_d)
            bv_sb = constp.tile([128, DG], F32)
            nc.sync.dma_start(bv_sb, bv_d)

            wq_sb = bigp.tile([128, KT, DG], BF16)
            nc.sync.dma_start(wq_sb, wqT_d.rearrange("(k p) d -> p k d", p=128))
            wk_sb = bigp.tile([128, KT, DG], BF16)
            nc.sync.dma_start(wk_sb, wkT_d.rearrange("(k p) d -> p k d", p=128))
            wv_sb = bigp.tile([128, KT, DG], BF16)
            nc.sync.dma_start(wv_sb, wvT_d.rearrange("(k p) d -> p k d", p=128))
            wo_sb = bigp.tile([128, DG // 128, H], BF16)
            nc.sync.dma_start(wo_sb, woT_d.rearrange("(k p) d -> p k d", p=128))

            qT = bigp.tile([128, 2, S], BF16)
            kTt = bigp.tile([128, 2, S], BF16)
            vN = bigp.tile([128, NT, DG], BF16)
            cT = bigp.tile([128, 2, S], BF16)
            mv_all = bigp.tile([128, NT, 2], F32)
            rstd_all = bigp.tile([128, NT], F32)

            # ---- pass A: load x (resident), row stats (mean/var over H) ----
            xts = []
            for i in range(NT):
                xt = xinp.tile([128, H], F32, tag=f"xt{i}", bufs=1)
                nc.sync.dma_start(xt, x_d[i * 128 : (i + 1) * 128, :])
                st = workp.tile([128, 2, 6], F32, tag="st")
                for a in range(2):
                    nc.vector.bn_stats(st[:, a, :], xt[:, a * 512 : (a + 1) * 512])
                nc.vector.bn_aggr(mv_all[:, i, :], st)
                xts.append(xt)
            # rstd = exp(-0.5 * ln(var + eps)); Ln/Exp batched -> few table loads
            lnv = workp.tile([128, NT], F32, tag="lnv")
            nc.scalar.activation(lnv, mv_all[:, :, 1], AF.Ln, bias=eps_b)
            nc.scalar.activation(rstd_all, lnv, AF.Exp, scale=-0.5)

            # ---- pass B: per column-chunk: normalize, transpose, QKV ----
            with tc.tile_pool(name="ph12", bufs=2) as zpool:
                for n in range(NC2):
                    zTc = zpool.tile([128, KT, CW], BF16, tag="zTc")
                    for i4 in range(CWT):
                        i = n * CWT + i4
                        zt = workp.tile([128, H], BF16, tag="zt", bufs=2)
                        nc.vector.tensor_scalar(
                            zt, xts[i], mv_all[:, i, 0:1], rstd_all[:, i : i + 1],
                            ALU.subtract, ALU.mult,
                        )
                        nc.sync.dma_start_transpose(
                            zTc[:, :, i4 * 128 : (i4 + 1) * 128], zt
                        )
                    if debug_outs:
                        nc.sync.dma_start(dbg_zT[:, :, n * CW : (n + 1) * CW], zTc)
                    # q/k for this chunk (transposed layout)
                    for tout, wsb, bsb in ((qT, wq_sb, bq_sb), (kTt, wk_sb, bk_sb)):
                        for m in range(2):
                            ps = psA.tile([128, max(HALF, 512)], F32,
                                          tag="sc", bufs=2)
                            for kk in range(KT):
                                nc.tensor.matmul(
                                    ps[:, 0:CW],
                                    wsb[:, kk, m * 128 : (m + 1) * 128],
                                    zTc[:, kk, :],
                                    start=(kk == 0),
                                    stop=(kk == KT - 1),
                                )
                            nc.vector.tensor_scalar_add(
                                tout[:, m, n * CW : (n + 1) * CW], ps[:, 0:CW],
                                bsb[:, m : m + 1],
                            )
                    # v for this chunk (natural layout)
                    for i4 in range(CWT):
                        i = n * CWT + i4
                        ps = psA.tile([128, max(HALF, 512)], F32, tag="sc",
                                      bufs=2)
                        for kk in range(KT):
                            nc.tensor.matmul(
                                ps[:, 0:DG],
                                zTc[:, kk, i4 * 128 : (i4 + 1) * 128],
                                wv_sb[:, kk, :],
                                start=(kk == 0),
                                stop=(kk == KT - 1),
                            )
                        nc.vector.tensor_tensor(vN[:, i, :], ps[:, 0:DG], bv_sb,
                                                ALU.add)

            if debug_outs:
                nc.sync.dma_start(dbg_qT, qT)
                nc.sync.dma_start(dbg_kT, kTt)
                nc.sync.dma_start(dbg_vN, vN)

            # ---- attention ----
            if phases < 2:
                return_early = True
            else:
                return_early = False
            for sH in range(2 if not return_early else 0):
                sq0 = sH * HALF
                for p in range(2):  # head pair = M-tile of qT/kT
                    ctx_ps = psB.tile([128, HALF], F32, tag="ctx")
                    sums_ps = psB.tile([128, 512], F32, tag="sums")

                    def emit_pv(jj, pjs):
                        for c in range(NCH):
                            for h in range(2):
                                nc.tensor.matmul(
                                    ctx_ps[64 * h : 64 * h + 64,
                                           c * CH : (c + 1) * CH],
                                    vN[:, jj, 64 * (2 * p + h) :
                                       64 * (2 * p + h) + 64],
                                    pjs[h][:, c * CH : (c + 1) * CH],
                                    tile_position=(0, 64 * h),
                                    start=(jj == 0),
                                    stop=(jj == NT - 1),
                                    skip_group_check=True,
                                )
                        for c in range(NCH):
                            for h in range(2):
                                pos = 64 * c + 32 * h
                                nc.tensor.matmul(
                                    sums_ps[pos : pos + 1, 0:CH],
                                    ones[:, 0:1],
                                    pjs[h][:, c * CH : (c + 1) * CH],
                                    tile_position=(0, pos),
                                    start=(jj == 0),
                                    stop=(jj == NT - 1),
                                    skip_group_check=True,
                                )

                    prev = None
                    for j in range(NT):
                        prs = []
                        for h in range(2):
                            sc = psA.tile([128, HALF], F32, tag="sc", bufs=2)
                            for c in range(NCH):
                                nc.tensor.matmul(
                                    sc[:, c * CH : (c + 1) * CH],
                                    kTt[64 * h : 64 * h + 64, p,
                                        j * 128 : (j + 1) * 128],
                                    qT[64 * h : 64 * h + 64, p,
                                       sq0 + c * CH : sq0 + (c + 1) * CH],
                                    tile_position=(64 * h, 0),
                                    start=True,
                                    stop=True,
                                )
                            pr = probsp.tile([128, HALF], BF16, tag=f"pr{h}")
                            nc.scalar.activation(
                                pr, sc, AF.Exp, bias=mask_sb[:, j : j + 1],
                                scale=0.125,
                            )
                            prs.append(pr)
                            if debug_outs and sH == 0 and p == 0 and j == 0 and h == 0:
                                nc.sync.dma_start(dbg_pr, pr)
                        # PV/sums for iteration j-1: keeps PE off the ACT
                        # critical path (scores for j+1 never wait on exp(j))
                        if prev is not None:
                            emit_pv(j - 1, prev)
                        prev = prs
                    emit_pv(NT - 1, prev)
                    if debug_outs and sH == 0 and p == 0:
                        sums_sb = workp.tile([128, 512], F32, tag="dbgs", bufs=1)
                        ctx_sb = workp.tile([128, HALF], F32, tag="dbgc", bufs=1)
                        for c in range(NCH):
                            for h in range(2):
                                pos = 64 * c + 32 * h
                                nc.vector.tensor_copy(
                                    sums_sb[pos : pos + 1, 0:CH],
                                    sums_ps[pos : pos + 1, 0:CH],
                                )
                        nc.vector.tensor_copy(ctx_sb, ctx_ps)
                        nc.sync.dma_start(dbg_sums, sums_sb)
                        nc.sync.dma_start(dbg_ctxps, ctx_sb)
                    sums_sb = workp.tile([128, 512], F32, tag="sums_sb", bufs=2)
                    recip_f = workp.tile([128, 512], F32, tag="recip_f", bufs=2)
                    recip = workp.tile([128, 512], BF16, tag="recip", bufs=2)
                    with nc.allow_low_precision("softmax recip in bf16"):
                        for c in range(NCH):
                            for h in range(2):
                                pos = 64 * c + 32 * h
                                nc.vector.tensor_copy(
                                    sums_sb[pos : pos + 1, 0:CH],
                                    sums_ps[pos : pos + 1, 0:CH],
                                )
                                nc.vector.reciprocal(
                                    recip_f[pos : pos + 1, 0:CH],
                                    sums_sb[pos : pos + 1, 0:CH],
                                )
                                nc.vector.tensor_copy(
                                    recip[pos : pos + 1, 0:CH],
                                    recip_f[pos : pos + 1, 0:CH],
                                )
                    rb_ps = psA.tile([128, HALF], F32, tag="sc", bufs=2)
                    for c in range(NCH):
                        for h in range(2):
                            pos = 64 * c + 32 * h
                            nc.tensor.matmul(
                                rb_ps[64 * h : 64 * h + 64, c * CH : (c + 1) * CH],
                                ones[pos : pos + 1, :],
                                recip[pos : pos + 1, 0:CH],
                                tile_position=(pos, 64 * h),
                                start=True,
                                stop=True,
                                skip_group_check=True,
                            )
                    rb = workp.tile([128, HALF], F32, tag="rb", bufs=2)
                    nc.vector.tensor_copy(rb, rb_ps)
                    if debug_outs and sH == 0 and p == 0:
                        rcf = workp.tile([128, 512], F32, tag="dbgr", bufs=1)
                        for c in range(NCH):
                            for h in range(2):
                                pos = 64 * c + 32 * h
                                nc.vector.tensor_copy(
                                    rcf[pos : pos + 1, 0:CH],
                                    recip_f[pos : pos + 1, 0:CH],
                                )
                        nc.sync.dma_start(dbg_recip, rcf)
                        nc.sync.dma_start(dbg_rb, rb)
                    nc.vector.tensor_tensor(
                        cT[:, p, sq0 : sq0 + HALF], ctx_ps, rb, ALU.mult
                    )

                # ---- output projection for this S_q half (overlaps the
                # next half's attention on ACT) ----
                for i in range(sH * NT // 2,
                               (sH + 1) * NT // 2 if phases >= 3 else 0):
                    ot = workp.tile([128, H], F32, tag="ot", bufs=2)
                    for n in range(H // 512):
                        ps = psA.tile([128, max(HALF, 512)], F32, tag="sc",
                                      bufs=2)
                        for kk in range(DG // 128):
                            nc.tensor.matmul(
                                ps[:, 0:512],
                                cT[:, kk, i * 128 : (i + 1) * 128],
                                wo_sb[:, kk, n * 512 : (n + 1) * 512],
                                start=(kk == 0),
                                stop=(kk == DG // 128 - 1),
                            )
                        nc.vector.tensor_copy(ot[:, n * 512 : (n + 1) * 512],
                                              ps[:, 0:512])
                    nc.sync.dma_start(out_d[i * 128 : (i + 1) * 128, :], ot)

            if debug_outs:
                nc.sync.dma_start(dbg_cT, cT)

    nc.compile()
    return nc


def make_in_maps(hidden_states, attention_mask, wq, bq, wk, bk, wv, bv, wo, bo,
                 ln_gamma, ln_beta, S):
    NT = S // 128
    g32 = np.asarray(ln_gamma).astype(np.float32)
    b32 = np.asarray(ln_beta).astype(np.float32)
    bf = ml_dtypes.bfloat16
    in_maps = []
    for c in range(NCORES):
        b = c // 4
        g = c % 4
        sl = slice(g * DG, (g + 1) * DG)
        wq_sl = np.asarray(wq)[sl, :].astype(np.float32)
        wk_sl = np.asarray(wk)[sl, :].astype(np.float32)
        wv_sl = np.asarray(wv)[sl, :].astype(np.float32)
        m = {
            "x": np.ascontiguousarray(np.asarray(hidden_states)[b], dtype=np.float32),
            "wqT": np.ascontiguousarray((wq_sl * g32[None, :]).T.astype(bf)),
            "wkT": np.ascontiguousarray((wk_sl * g32[None, :]).T.astype(bf)),
            "wvT": np.ascontiguousarray((wv_sl * g32[None, :]).T.astype(bf)),
            "woT": np.ascontiguousarray(
                np.asarray(wo)[:, sl].astype(np.float32).T.astype(bf)
            ),
            "bq": np.ascontiguousarray(
                (wq_sl @ b32 + np.asarray(bq)[sl]).astype(np.float32).reshape(2, 128).T
            ),
            "bk": np.ascontiguousarray(
                (wk_sl @ b32 + np.asarray(bk)[sl]).astype(np.float32).reshape(2, 128).T
            ),
            "bv": np.ascontiguousarray(
                np.broadcast_to(
                    (wv_sl @ b32 + np.asarray(bv)[sl]).astype(np.float32), (128, DG)
                ).copy()
            ),
            "mask": np.ascontiguousarray(
                np.asarray(attention_mask)[b, 0, 0, :]
                .astype(np.float32).reshape(NT, 128).T
            ),
        }
        in_maps.append(m)
    return in_maps


_NC_CACHE = {}


def kernel(hidden_states, attention_mask, wq, bq, wk, bk, wv, bv, wo, bo,
           ln_gamma, ln_beta):
    hidden_states = np.asarray(hidden_states)
    B, S, _ = hidden_states.shape
    if S not in _NC_CACHE:
        _NC_CACHE[S] = build_program(S)
    nc = _NC_CACHE[S]

    in_maps = make_in_maps(
        hidden_states, attention_mask, wq, bq, wk, bk, wv, bv, wo, bo,
        ln_gamma, ln_beta, S,
    )

    from concourse.bass_utils import run_bass_kernel_spmd

    res = run_bass_kernel_spmd(nc, in_maps, list(range(NCORES)))
    parts = [res.results[c]["out"] for c in range(NCORES)]

    out = np.empty((B, S, H), np.float32)
    bo32 = np.asarray(bo).astype(np.float32)
    for b in range(B):
        acc = parts[4 * b].astype(np.float32).copy()
        for g in range(1, 4):
            acc += parts[4 * b + g]
        out[b] = acc + bo32[None, :] + hidden_states[b].astype(np.float32)
    return out


# revision 26
# speedup vs baseline: 12945.6854x; 1.0410x over previous
"""Fused pre-LN multi-head attention block for Trainium2, sharded over 8 NeuronCores.

Sharding: batch x head-group tensor parallel. Core c handles batch b=c//4 and
head group g=c%4 (4 heads of 64 dims = 256 columns of Wq/Wk/Wv, 256 rows of Wo).
Each core computes LayerNorm(x_b) (gamma/beta folded into weights host-side),
QKV for its heads, attention, and a partial output projection. The host sums
the 4 partials per batch and adds bias + residual (the attention branch is tiny
next to the residual, so bf16 matmul inputs cost ~1e-5 scale-relative error).

Device pipeline (per core):
  pass A: load x tiles (kept resident), row stats, batched rstd=exp(-.5*ln(var+eps))
  pass B, per 512-col chunk of S: normalize to bf16, DMA-xbar transpose into
    zTc [H part, 512], then QKV matmuls (q/k transposed layout [256 part, S],
    v natural [S part, 256]).
  attention, per (S_q half, head pair): per S_k tile j, row-tiled pair matmul
    -> scoresT [128, half] PSUM -> exp on ACT (mask/scale folded in) -> probsT
    bf16 SBUF; col-tiled PV accumulates ctxT in PSUM; ones-matmul row sums;
    softmax normalization folded into ctx eviction via partition_broadcast.
  output projection: ctxT^T @ woT -> partial out [S, H] fp32.
"""

import os
import sys

sys.path.insert(0, "/opt/trn_rl_repo")

import numpy as np
import ml_dtypes

import concourse.bacc as bacc
import concourse.bass as bass
import concourse.mybir as mybir
from concourse import tile

F32 = mybir.dt.float32
BF16 = mybir.dt.bfloat16
AF = mybir.ActivationFunctionType
ALU = mybir.AluOpType

H = 1024
NHEADS = 16
HD = 64
DG = 256  # head dims per core (4 heads x 64)
NCORES = 8
EPS = 1e-12


def build_program(S=2048, debug_outs=False, phases=3):
    nc = bacc.Bacc(
        "TRN2", target_bir_lowering=False, debug=False, num_devices=NCORES
    )
    x_d = nc.dram_tensor("x", [S, H], F32, kind="ExternalInput").ap()
    wqT_d = nc.dram_tensor("wqT", [H, DG], BF16, kind="ExternalInput").ap()
    wkT_d = nc.dram_tensor("wkT", [H, DG], BF16, kind="ExternalInput").ap()
    wvT_d = nc.dram_tensor("wvT", [H, DG], BF16, kind="ExternalInput").ap()
    woT_d = nc.dram_tensor("woT", [DG, H], BF16, kind="ExternalInput").ap()
    bq_d = nc.dram_tensor("bq", [128, 2], F32, kind="ExternalInput").ap()
    bk_d = nc.dram_tensor("bk", [128, 2], F32, kind="ExternalInput").ap()
    bv_d = nc.dram_tensor("bv", [128, DG], F32, kind="ExternalInput").ap()
    mask_d = nc.dram_tensor("mask", [128, S // 128], F32, kind="ExternalInput").ap()
    out_d = nc.dram_tensor("out", [S, H], F32, kind="ExternalOutput").ap()
    if debug_outs:
        dbg_zT = nc.dram_tensor("dbg_zT", [128, H // 128, S], BF16, kind="ExternalOutput").ap()
        dbg_qT = nc.dram_tensor("dbg_qT", [128, 2, S], BF16, kind="ExternalOutput").ap()
        dbg_kT = nc.dram_tensor("dbg_kT", [128, 2, S], BF16, kind="ExternalOutput").ap()
        dbg_vN = nc.dram_tensor("dbg_vN", [128, S // 128, DG], BF16, kind="ExternalOutput").ap()
        dbg_cT = nc.dram_tensor("dbg_cT", [128, 2, S], BF16, kind="ExternalOutput").ap()
        dbg_pr = nc.dram_tensor("dbg_pr", [128, S // 2], BF16, kind="ExternalOutput").ap()
        dbg_sums = nc.dram_tensor("dbg_sums", [128, 512], F32, kind="ExternalOutput").ap()
        dbg_recip = nc.dram_tensor("dbg_recip", [128, 512], F32, kind="ExternalOutput").ap()
        dbg_rb = nc.dram_tensor("dbg_rb", [128, S // 2], F32, kind="ExternalOutput").ap()
        dbg_ctxps = nc.dram_tensor("dbg_ctxps", [128, S // 2], F32, kind="ExternalOutput").ap()

    NT = S // 128  # S tiles
    KT = H // 128  # 8
    HALF = S // 2  # S_q half width
    CH = min(512, HALF)  # S_q chunk (one PSUM bank)
    NCH = HALF // CH  # chunks per half
    CW = min(512, S)  # column chunk for pass B
    NC2 = S // CW
    CWT = CW // 128  # S tiles per chunk

    with tile.TileContext(nc) as tc:
        with (
            tc.tile_pool(name="const", bufs=1) as constp,
            tc.tile_pool(name="big", bufs=1) as bigp,
            tc.tile_pool(name="xin", bufs=1) as xinp,
            tc.tile_pool(name="work", bufs=3) as workp,
            tc.tile_pool(name="probs", bufs=3) as probsp,
            tc.tile_pool(name="psA", bufs=2, space="PSUM") as psA,
            tc.tile_pool(name="psB", bufs=1, space="PSUM") as psB,
        ):
            ones_f = constp.tile([128, 64], F32)
            nc.gpsimd.memset(ones_f, 1.0)
            ones = constp.tile([128, 64], BF16)
            nc.vector.tensor_copy(ones, ones_f)
            eps_b = constp.tile([128, 1], F32)
            nc.gpsimd.memset(eps_b, EPS)
            mask_sb = constp.tile([128, NT], F32)
            nc.sync.dma_start(mask_sb, mask_d)
            bq_sb = constp.tile([128, 2], F32)
            nc.sync.dma_start(bq_sb, bq_d)
            bk_sb = constp.tile([128, 2], F32)
            nc.sync.dma_start(bk_sb, bk_d)
            bv_sb = constp.tile([128, DG], F32)
            nc.sync.dma_start(bv_sb, bv_d)

            wq_sb = bigp.tile([128, KT, DG], BF16)
            nc.sync.dma_start(wq_sb, wqT_d.rearrange("(k p) d -> p k d", p=128))
            wk_sb = bigp.tile([128, KT, DG], BF16)
            nc.sync.dma_start(wk_sb, wkT_d.rearrange("(k p) d -> p k d", p=128))
            wv_sb = bigp.tile([128, KT, DG], BF16)
            nc.sync.dma_start(wv_sb, wvT_d.rearrange("(k p) d -> p k d", p=128))
            wo_sb = bigp.tile([128, DG // 128, H], BF16)
            nc.sync.dma_start(wo_sb, woT_d.rearrange("(k p) d -> p k d", p=128))

            qT = bigp.tile([128, 2, S], BF16)
            kTt = bigp.tile([128, 2, S], BF16)
            vN = bigp.tile([128, NT, DG], BF16)
            cT = bigp.tile([128, 2, S], BF16)
            mv_all = bigp.tile([128, NT, 2], F32)
            rstd_all = bigp.tile([128, NT], F32)

            # ---- pass A: load x (resident), row stats (mean/var over H) ----
            xts = []
            for i in range(NT):
                xt = xinp.tile([128, H], F32, tag=f"xt{i}", bufs=1)
                nc.sync.dma_start(xt, x_d[i * 128 : (i + 1) * 128, :])
                st = workp.tile([128, 2, 6], F32, tag="st")
                for a in range(2):
                    nc.vector.bn_stats(st[:, a, :], xt[:, a * 512 : (a + 1) * 512])
                nc.vector.bn_aggr(mv_all[:, i, :], st)
                xts.append(xt)
            # rstd = exp(-0.5 * ln(var + eps)); Ln/Exp batched -> few table loads
            lnv = workp.tile([128, NT], F32, tag="lnv")
            nc.scalar.activation(lnv, mv_all[:, :, 1], AF.Ln, bias=eps_b)
            nc.scalar.activation(rstd_all, lnv, AF.Exp, scale=-0.5)

            # ---- pass B: per column-chunk: normalize, transpose, QKV ----
            with tc.tile_pool(name="ph12", bufs=2) as zpool:
                for n in range(NC2):
                    zTc = zpool.tile([128, KT, CW], BF16, tag="zTc")
                    for i4 in range(CWT):
                        i = n * CWT + i4
                        zt = workp.tile([128, H], BF16, tag="zt", bufs=2)
                        nc.vector.tensor_scalar(
                            zt, xts[i], mv_all[:, i, 0:1], rstd_all[:, i : i + 1],
                            ALU.subtract, ALU.mult,
                        )
                        nc.sync.dma_start_transpose(
                            zTc[:, :, i4 * 128 : (i4 + 1) * 128], zt
                        )
                    if debug_outs:
                        nc.sync.dma_start(dbg_zT[:, :, n * CW : (n + 1) * CW], zTc)
                    # q/k for this chunk (transposed layout)
                    for tout, wsb, bsb in ((qT, wq_sb, bq_sb), (kTt, wk_sb, bk_sb)):
                        for m in range(2):
                            ps = psA.tile([128, max(HALF, 512)], F32,
                                          tag="sc", bufs=2)
                            for kk in range(KT):
                                nc.tensor.matmul(
                                    ps[:, 0:CW],
                                    wsb[:, kk, m * 128 : (m + 1) * 128],
                                    zTc[:, kk, :],
                                    start=(kk == 0),
                                    stop=(kk == KT - 1),
                                )
                            nc.vector.tensor_scalar_add(
                                tout[:, m, n * CW : (n + 1) * CW], ps[:, 0:CW],
                                bsb[:, m : m + 1],
                            )
                    # v for this chunk (natural layout)
                    for i4 in range(CWT):
                        i = n * CWT + i4
                        ps = psA.tile([128, max(HALF, 512)], F32, tag="sc",
                                      bufs=2)
                        for kk in range(KT):
                            nc.tensor.matmul(
                                ps[:, 0:DG],
                                zTc[:, kk, i4 * 128 : (i4 + 1) * 128],
                                wv_sb[:, kk, :],
                                start=(kk == 0),
                                stop=(kk == KT - 1),
                            )
                        nc.vector.tensor_tensor(vN[:, i, :], ps[:, 0:DG], bv_sb,
                                                ALU.add)

            if debug_outs:
                nc.sync.dma_start(dbg_qT, qT)
                nc.sync.dma_start(dbg_kT, kTt)
                nc.sync.dma_start(dbg_vN, vN)

            # ---- attention ----
            if phases < 2:
                return_early = True
            else:
                return_early = False
            for sH in range(2 if not return_early else 0):
                sq0 = sH * HALF
                for p in range(2):  # head pair = M-tile of qT/kT
                    ctx_ps = psB.tile([128, HALF], F32, tag="ctx")
                    sums_ps = psB.tile([128, 512], F32, tag="sums")

                    def emit_pv(jj, pjs):
                        for c in range(NCH):
                            for h in range(2):
                                nc.tensor.matmul(
                                    ctx_ps[64 * h : 64 * h + 64,
                                           c * CH : (c + 1) * CH],
                                    vN[:, jj, 64 * (2 * p + h) :
                                       64 * (2 * p + h) + 64],
                                    pjs[h][:, c * CH : (c + 1) * CH],
                                    tile_position=(0, 64 * h),
                                    start=(jj == 0),
                                    stop=(jj == NT - 1),
                                    skip_group_check=True,
                                )
                        for c in range(NCH):
                            for h in range(2):
                                pos = 64 * c + 32 * h
                                nc.tensor.matmul(
                                    sums_ps[pos : pos + 1, 0:CH],
                                    ones[:, 0:1],
                                    pjs[h][:, c * CH : (c + 1) * CH],
                                    tile_position=(0, pos),
                                    start=(jj == 0),
                                    stop=(jj == NT - 1),
                                    skip_group_check=True,
                                )

                    prev = None
                    for j in range(NT):
                        prs = []
                        for h in range(2):
                            sc = psA.tile([128, HALF], F32, tag="sc", bufs=2)
                            for c in range(NCH):
                                nc.tensor.matmul(
                                    sc[:, c * CH : (c + 1) * CH],
                                    kTt[64 * h : 64 * h + 64, p,
                                        j * 128 : (j + 1) * 128],
                                    qT[64 * h : 64 * h + 64, p,
                                       sq0 + c * CH : sq0 + (c + 1) * CH],
                                    tile_position=(64 * h, 0),
                                    start=True,
                                    stop=True,
                                )
                            pr = probsp.tile([128, HALF], BF16, tag=f"pr{h}")
                            nc.scalar.activation(
                                pr, sc, AF.Exp, bias=mask_sb[:, j : j + 1],
                                scale=0.125,
                            )
                            prs.append(pr)
                            if debug_outs and sH == 0 and p == 0 and j == 0 and h == 0:
                                nc.sync.dma_start(dbg_pr, pr)
                        # PV/sums for iteration j-1: keeps PE off the ACT
                        # critical path (scores for j+1 never wait on exp(j))
                        if prev is not None:
                            emit_pv(j - 1, prev)
                        prev = prs
                    emit_pv(NT - 1, prev)
                    if debug_outs and sH == 0 and p == 0:
                        sums_sb = workp.tile([128, 512], F32, tag="dbgs", bufs=1)
                        ctx_sb = workp.tile([128, HALF], F32, tag="dbgc", bufs=1)
                        for c in range(NCH):
                            for h in range(2):
                                pos = 64 * c + 32 * h
                                nc.vector.tensor_copy(
                                    sums_sb[pos : pos + 1, 0:CH],
                                    sums_ps[pos : pos + 1, 0:CH],
                                )
                        nc.vector.tensor_copy(ctx_sb, ctx_ps)
                        nc.sync.dma_start(dbg_sums, sums_sb)
                        nc.sync.dma_start(dbg_ctxps, ctx_sb)
                    # whole-tile ops: unused partition rows hold stale PSUM
                    # data whose reciprocal is never read by the broadcast
                    sums_sb = workp.tile([128, 512], F32, tag="sums_sb", bufs=2)
                    recip_f = workp.tile([128, 512], F32, tag="recip_f", bufs=2)
                    recip = workp.tile([128, 512], BF16, tag="recip", bufs=2)
                    with nc.allow_low_precision("softmax recip in bf16"):
                        nc.vector.tensor_copy(sums_sb, sums_ps)
                        nc.vector.reciprocal(recip_f, sums_sb)
                        nc.vector.tensor_copy(recip, recip_f)
                    rb_ps = psA.tile([128, HALF], F32, tag="sc", bufs=2)
                    for c in range(NCH):
                        for h in range(2):
                            pos = 64 * c + 32 * h
                            nc.tensor.matmul(
                                rb_ps[64 * h : 64 * h + 64, c * CH : (c + 1) * CH],
                                ones[pos : pos + 1, :],
                                recip[pos : pos + 1, 0:CH],
                                tile_position=(pos, 64 * h),
                                start=True,
                                stop=True,
                                skip_group_check=True,
                            )
                    rb = workp.tile([128, HALF], F32, tag="rb", bufs=2)
                    nc.vector.tensor_copy(rb, rb_ps)
                    if debug_outs and sH == 0 and p == 0:
                        rcf = workp.tile([128, 512], F32, tag="dbgr", bufs=1)
                        for c in range(NCH):
                            for h in range(2):
                                pos = 64 * c + 32 * h
                                nc.vector.tensor_copy(
                                    rcf[pos : pos + 1, 0:CH],
                                    recip_f[pos : pos + 1, 0:CH],
                                )
                        nc.sync.dma_start(dbg_recip, rcf)
                        nc.sync.dma_start(dbg_rb, rb)
                    nc.vector.tensor_tensor(
                        cT[:, p, sq0 : sq0 + HALF], ctx_ps, rb, ALU.mult
                    )

                # ---- output projection for this S_q half (overlaps the
                # next half's attention on ACT) ----
                for i in range(sH * NT // 2,
                               (sH + 1) * NT // 2 if phases >= 3 else 0):
                    ot = workp.tile([128, H], F32, tag="ot", bufs=2)
                    for n in range(H // 512):
                        ps = psA.tile([128, max(HALF, 512)], F32, tag="sc",
                                      bufs=2)
                        for kk in range(DG // 128):
                            nc.tensor.matmul(
                                ps[:, 0:512],
                                cT[:, kk, i * 128 : (i + 1) * 128],
                                wo_sb[:, kk, n * 512 : (n + 1) * 512],
                                start=(kk == 0),
                                stop=(kk == DG // 128 - 1),
                            )
                        nc.vector.tensor_copy(ot[:, n * 512 : (n + 1) * 512],
                                              ps[:, 0:512])
                    nc.sync.dma_start(out_d[i * 128 : (i + 1) * 128, :], ot)

            if debug_outs:
                nc.sync.dma_start(dbg_cT, cT)

    nc.compile()
    return nc


def make_in_maps(hidden_states, attention_mask, wq, bq, wk, bk, wv, bv, wo, bo,
                 ln_gamma, ln_beta, S):
    NT = S // 128
    g32 = np.asarray(ln_gamma).astype(np.float32)
    b32 = np.asarray(ln_beta).astype(np.float32)
    bf = ml_dtypes.bfloat16
    in_maps = []
    for c in range(NCORES):
        b = c // 4
        g = c % 4
        sl = slice(g * DG, (g + 1) * DG)
        wq_sl = np.asarray(wq)[sl, :].astype(np.float32)
        wk_sl = np.asarray(wk)[sl, :].astype(np.float32)
        wv_sl = np.asarray(wv)[sl, :].astype(np.float32)
        m = {
            "x": np.ascontiguousarray(np.asarray(hidden_states)[b], dtype=np.float32),
            "wqT": np.ascontiguousarray((wq_sl * g32[None, :]).T.astype(bf)),
            "wkT": np.ascontiguousarray((wk_sl * g32[None, :]).T.astype(bf)),
            "wvT": np.ascontiguousarray((wv_sl * g32[None, :]).T.astype(bf)),
            "woT": np.ascontiguousarray(
                np.asarray(wo)[:, sl].astype(np.float32).T.astype(bf)
            ),
            "bq": np.ascontiguousarray(
                (wq_sl @ b32 + np.asarray(bq)[sl]).astype(np.float32).reshape(2, 128).T
            ),
            "bk": np.ascontiguousarray(
                (wk_sl @ b32 + np.asarray(bk)[sl]).astype(np.float32).reshape(2, 128).T
            ),
            "bv": np.ascontiguousarray(
                np.broadcast_to(
                    (wv_sl @ b32 + np.asarray(bv)[sl]).astype(np.float32), (128, DG)
                ).copy()
            ),
            "mask": np.ascontiguousarray(
                np.asarray(attention_mask)[b, 0, 0, :]
                .astype(np.float32).reshape(NT, 128).T
            ),
        }
        in_maps.append(m)
    return in_maps


_NC_CACHE = {}


def kernel(hidden_states, attention_mask, wq, bq, wk, bk, wv, bv, wo, bo,
           ln_gamma, ln_beta):
    hidden_states = np.asarray(hidden_states)
    B, S, _ = hidden_states.shape
    if S not in _NC_CACHE:
        _NC_CACHE[S] = build_program(S)
    nc = _NC_CACHE[S]

    in_maps = make_in_maps(
        hidden_states, attention_mask, wq, bq, wk, bk, wv, bv, wo, bo,
        ln_gamma, ln_beta, S,
    )

    from concourse.bass_utils import run_bass_kernel_spmd

    res = run_bass_kernel_spmd(nc, in_maps, list(range(NCORES)))
    parts = [res.results[c]["out"] for c in range(NCORES)]

    out = np.empty((B, S, H), np.float32)
    bo32 = np.asarray(bo).astype(np.float32)
    for b in range(B):
        acc = parts[4 * b].astype(np.float32).copy()
        for g in range(1, 4):
            acc += parts[4 * b + g]
        out[b] = acc + bo32[None, :] + hidden_states[b].astype(np.float32)
    return out
